# revision 1
# baseline (speedup 1.0000x reference)
"""Cross-attention Trainium2 kernel (8 NeuronCores, data-parallel over B x T-halves).

Problem (hardcoded): B=4, T=2048, M=4096, D=512, H=8, Dh=64, fp32 I/O.
Each of the 8 cores handles one (batch, T-half): x-shard [1024, 512], mem [4096, 512].

Per-core pipeline:
  1. PE-transpose mem chunks -> memT (D-major, fp32), project K^T (fp32r) and V
     (bf16, with a fused ones-column per head for the softmax denominator).
  2. Same for x -> Q^T (fp32r, head-major).
  3. Flash-style attention per head-pair: row-packed concurrent K=64 score
     matmuls -> PSUM, one large Exp per 3-bank PSUM group on ScalarE
     (bf16 out, scale=1/8), AV accumulation in bf16; denominator comes out of
     the ones-column; normalize with reciprocal + tensor_scalar.
  4. PE-transpose z, O-projection (fp32r), biases via K=1 matmuls.
"""

import numpy as np

B, T, M, D = 4, 2048, 4096, 512
H, DH = 8, 64
N_CORES = 8
TS = T // 2           # per-core T shard = 1024
ND = D // 128         # 4 d-tiles
NM = M // 128         # 32 m-tiles
NTT = TS // 128       # 8 t-tiles
TCH = 256             # attention t-chunk
NTC = TS // TCH       # 4 t-chunks
MGROUPS = [3, 3, 3, 3, 3, 3, 3, 3, 3, 3, 2]  # m-tiles per PSUM score group (sum=32)

_CACHE = {}


def _build():
    import concourse.bacc as bacc
    import concourse.mybir as mybir
    import concourse.tile as tile
    from contextlib import ExitStack

    f32 = mybir.dt.float32
    f16 = mybir.dt.float16
    bf16 = mybir.dt.bfloat16
    AF = mybir.ActivationFunctionType

    nc = bacc.Bacc("TRN2", target_bir_lowering=False, debug=False,
                   num_devices=N_CORES)

    x_d = nc.dram_tensor("x", [TS, D], f32, kind="ExternalInput").ap()
    mem_d = nc.dram_tensor("mem", [M, D], f32, kind="ExternalInput").ap()
    w_d = {}
    b_d = {}
    for nm in ("q", "k", "v", "o"):
        w_d[nm] = nc.dram_tensor(f"W{nm}", [D, D], f16, kind="ExternalInput").ap()
        b_d[nm] = nc.dram_tensor(f"b{nm}", [1, D], f16, kind="ExternalInput").ap()
    ident_d = nc.dram_tensor("ident", [128, 128], f32, kind="ExternalInput").ap()
    ones_d = nc.dram_tensor("ones", [1, D], f16, kind="ExternalInput").ap()
    out_d = nc.dram_tensor("out", [TS, D], f32, kind="ExternalOutput").ap()

    with tile.TileContext(nc) as tc, ExitStack() as top:
        const = top.enter_context(tc.tile_pool(name="const", bufs=1))
        persist = top.enter_context(tc.tile_pool(name="persist", bufs=1))

        ident = const.tile([128, 128], f32, tag="ident")
        nc.sync.dma_start(ident[:], ident_d[:])
        ones = const.tile([1, D], f16, tag="ones")
        nc.sync.dma_start(ones[:], ones_d[:])
        w_sb = {}
        b_sb = {}
        for nm in ("q", "k", "v", "o"):
            b_sb[nm] = const.tile([1, D], f16, tag=f"b{nm}", name=f"b{nm}")
            nc.sync.dma_start(b_sb[nm][:], b_d[nm][:])
            w_sb[nm] = [const.tile([128, D], f16, tag=f"W{nm}{di}", name=f"W{nm}{di}") for di in range(ND)]
            for di in range(ND):
                nc.sync.dma_start(w_sb[nm][di][:], w_d[nm][di * 128:(di + 1) * 128, :])

        # persistent activations
        KT = [persist.tile([128, M], f16, tag=f"KT{do}", name=f"KT{do}") for do in range(ND)]
        QT = [persist.tile([128, TS], f16, tag=f"QT{do}", name=f"QT{do}") for do in range(ND)]
        VX = [persist.tile([128, H * 65], f16, tag=f"VX{mt}", name=f"VX{mt}") for mt in range(NM)]
        ZSB = [persist.tile([128, D], f32, tag=f"Z{tt}", name=f"Z{tt}") for tt in range(NTT)]

        # ---- Phase 1: mem -> memT chunks -> K^T, V ----
        with (
            tc.tile_pool(name="nat", bufs=6) as nat_pool,
            tc.tile_pool(name="mt", bufs=8) as mt_pool,
            tc.tile_pool(name="ptr", bufs=2, space="PSUM") as ptr_pool,
            tc.tile_pool(name="pk", bufs=2, space="PSUM") as pk_pool,
            tc.tile_pool(name="pv", bufs=2, space="PSUM") as pv_pool,
        ):
            for mc in range(M // 512):  # 8 chunks of 512 rows of mem
                nat = []
                for j in range(4):
                    t = nat_pool.tile([128, D], f32, tag="nat")
                    nc.sync.dma_start(t[:], mem_d[(mc * 4 + j) * 128:(mc * 4 + j + 1) * 128, :])
                    nat.append(t)
                mT = []
                for di in range(ND):
                    ptr = ptr_pool.tile([128, 512], f32, tag="ptr")
                    for j in range(4):
                        nc.tensor.transpose(ptr[:, j * 128:(j + 1) * 128],
                                            nat[j][:, di * 128:(di + 1) * 128], ident[:])
                    mTt = mt_pool.tile([128, 512], f16, tag="mT")
                    nc.vector.tensor_copy(mTt[:], ptr[:])
                    mT.append(mTt)
                # K^T chunk: out [dout-128, 512 m-cols]
                for do in range(ND):
                    pk = pk_pool.tile([128, 512], f32, tag="pk")
                    for di in range(ND):
                        nc.tensor.matmul(
                            pk[:], w_sb["k"][di][:, do * 128:(do + 1) * 128],
                            mT[di][:], start=(di == 0), stop=False)
                    nc.tensor.matmul(pk[:], b_sb["k"][0:1, do * 128:(do + 1) * 128],
                                     ones[0:1, 0:512], start=False, stop=True)
                    nc.scalar.copy(KT[do][:, mc * 512:(mc + 1) * 512], pk[:])
                # V chunk: per m-tile [m-128, D] -> VX bf16 with ones cols
                for j in range(4):
                    mt = mc * 4 + j
                    pv = pv_pool.tile([128, 512], f32, tag="pv")
                    for di in range(ND):
                        nc.tensor.matmul(pv[:], mT[di][:, j * 128:(j + 1) * 128],
                                         w_sb["v"][di][:],
                                         start=(di == 0), stop=False)
                    nc.tensor.matmul(pv[:], ones[0:1, 0:128],
                                     b_sb["v"][:], start=False, stop=True)
                    vx3 = VX[mt][:].rearrange("p (h c) -> p h c", h=H)
                    nc.vector.tensor_copy(vx3[:, :, 0:64],
                                          pv[:].rearrange("p (h c) -> p h c", h=H))
                    nc.vector.memset(vx3[:, :, 64:65], 1.0)

        # ---- Phase 2: x -> xT chunks -> Q^T ----
        with (
            tc.tile_pool(name="natx", bufs=6) as natx_pool,
            tc.tile_pool(name="xt", bufs=8) as xt_pool,
            tc.tile_pool(name="ptrx", bufs=2, space="PSUM") as ptrx_pool,
            tc.tile_pool(name="pq", bufs=2, space="PSUM") as pq_pool,
        ):
            for tcx in range(TS // 512):  # 2 chunks
                nat = []
                for j in range(4):
                    t = natx_pool.tile([128, D], f32, tag="natx")
                    nc.sync.dma_start(t[:], x_d[(tcx * 4 + j) * 128:(tcx * 4 + j + 1) * 128, :])
                    nat.append(t)
                xT = []
                for di in range(ND):
                    ptr = ptrx_pool.tile([128, 512], f32, tag="ptrx")
                    for j in range(4):
                        nc.tensor.transpose(ptr[:, j * 128:(j + 1) * 128],
                                            nat[j][:, di * 128:(di + 1) * 128], ident[:])
                    xTt = xt_pool.tile([128, 512], f16, tag="xT")
                    nc.vector.tensor_copy(xTt[:], ptr[:])
                    xT.append(xTt)
                for do in range(ND):
                    pq = pq_pool.tile([128, 512], f32, tag="pq")
                    for di in range(ND):
                        nc.tensor.matmul(
                            pq[:], w_sb["q"][di][:, do * 128:(do + 1) * 128],
                            xT[di][:], start=(di == 0), stop=False)
                    nc.tensor.matmul(pq[:], b_sb["q"][0:1, do * 128:(do + 1) * 128],
                                     ones[0:1, 0:512], start=False, stop=True)
                    nc.scalar.copy(QT[do][:, tcx * 512:(tcx + 1) * 512], pq[:])

        # ---- Phase 3: attention per (head-pair, t-chunk) ----
        with (
            tc.tile_pool(name="psc", bufs=2, space="PSUM") as psc_pool,
            tc.tile_pool(name="pz", bufs=2, space="PSUM") as pz_pool,
            tc.tile_pool(name="esb", bufs=13) as e_pool,
            tc.tile_pool(name="rcp", bufs=2) as rcp_pool,
        ):
            for hp in range(H // 2):
                for tci in range(NTC):
                    pz = pz_pool.tile([128, 4 * 65], f32, tag="pz")
                    egroups = []
                    mt0 = 0
                    HB = 3 * TCH  # per-head column block inside a score group
                    for msz in MGROUPS:
                        # layout [128, 1536]: h0 cols [0, msz*256), h1 at [768, ...)
                        # -> the (h0, h1) MM pair of each m-tile lands in
                        #    different PSUM banks (concurrent disjoint row groups)
                        psc = psc_pool.tile([128, 3 * 512], f32, tag="psc")
                        for j in range(msz):
                            mt = mt0 + j
                            for hl in range(2):
                                nc.tensor.matmul(
                                    psc[:, hl * HB + j * TCH: hl * HB + (j + 1) * TCH],
                                    KT[hp][hl * 64:(hl + 1) * 64,
                                           mt * 128:(mt + 1) * 128],
                                    QT[hp][hl * 64:(hl + 1) * 64,
                                           tci * TCH:(tci + 1) * TCH],
                                    start=True, stop=True,
                                    tile_position=(hl * 64, 0))
                        esb = e_pool.tile([128, 3 * 512], f16, tag="esb")
                        # always exp the full contiguous 3-bank group: for the
                        # ragged last group the unwritten columns hold stale
                        # scores whose exp is never read (and cannot overflow)
                        nc.scalar.activation(esb[:], psc[:], AF.Exp, scale=0.125)
                        egroups.append((mt0, msz, esb))
                        mt0 += msz
                    # AV: each (ts, hl) PSUM accumulation chain runs as
                    # consecutive matmuls -- interleaved open accumulation
                    # chains corrupt PSUM (only the last-issued one survives)
                    for ts in range(2):
                        for hl in range(2):
                            for g0, gsz, esb in egroups:
                                for j in range(gsz):
                                    mt = g0 + j
                                    nc.tensor.matmul(
                                        pz[:, (ts * 2 + hl) * 65:(ts * 2 + hl) * 65 + 65],
                                        esb[:, hl * HB + j * TCH + ts * 128:
                                            hl * HB + j * TCH + (ts + 1) * 128],
                                        VX[mt][:].rearrange("p (h c) -> p h c", h=H)[:, 2 * hp + hl, :],
                                        start=(mt == 0), stop=(mt == NM - 1))
                    rcp = rcp_pool.tile([128, 4], f32, tag="rcp")
                    nc.vector.reciprocal(
                        rcp[:], pz[:].rearrange("p (k c) -> p k c", c=65)[:, :, 64:65])
                    for ts in range(2):
                        tt = tci * 2 + ts
                        for hl in range(2):
                            nc.vector.tensor_scalar_mul(
                                ZSB[tt][:, hp * 128 + hl * 64: hp * 128 + (hl + 1) * 64],
                                pz[:, (ts * 2 + hl) * 65:(ts * 2 + hl) * 65 + 64],
                                rcp[:, ts * 2 + hl: ts * 2 + hl + 1])

        # ---- Phase 4: O-projection ----
        with (
            tc.tile_pool(name="pzt", bufs=2, space="PSUM") as pzt_pool,
            tc.tile_pool(name="po", bufs=2, space="PSUM") as po_pool,
            tc.tile_pool(name="zt", bufs=3) as zt_pool,
            tc.tile_pool(name="ob", bufs=3) as ob_pool,
        ):
            for tt in range(NTT):
                pzt = pzt_pool.tile([128, 512], f32, tag="pzt")
                for di in range(ND):
                    nc.tensor.transpose(pzt[:, di * 128:(di + 1) * 128],
                                        ZSB[tt][:, di * 128:(di + 1) * 128], ident[:])
                zT = zt_pool.tile([128, 512], f16, tag="zT")
                nc.vector.tensor_copy(zT[:], pzt[:])
                po = po_pool.tile([128, 512], f32, tag="po")
                for di in range(ND):
                    nc.tensor.matmul(po[:], zT[:, di * 128:(di + 1) * 128],
                                     w_sb["o"][di][:],
                                     start=(di == 0), stop=False)
                nc.tensor.matmul(po[:], ones[0:1, 0:128],
                                 b_sb["o"][:], start=False, stop=True)
                osb = ob_pool.tile([128, 512], f32, tag="osb")
                nc.scalar.copy(osb[:], po[:])
                nc.sync.dma_start(out_d[tt * 128:(tt + 1) * 128, :], osb[:])

    nc.finalize()
    return nc


def _get_nc():
    if "nc" not in _CACHE:
        _CACHE["nc"] = _build()
    return _CACHE["nc"]


def build_in_maps(x, mem, Wq, bq, Wk, bk, Wv, bv, Wo, bo, **kw):
    x = np.ascontiguousarray(np.asarray(x, np.float32))
    mem = np.ascontiguousarray(np.asarray(mem, np.float32))
    common = {
        "Wq": np.ascontiguousarray(np.asarray(Wq, np.float32).astype(np.float16)),
        "Wk": np.ascontiguousarray(np.asarray(Wk, np.float32).astype(np.float16)),
        "Wv": np.ascontiguousarray(np.asarray(Wv, np.float32).astype(np.float16)),
        "Wo": np.ascontiguousarray(np.asarray(Wo, np.float32).astype(np.float16)),
        "bq": np.asarray(bq, np.float32).astype(np.float16).reshape(1, D),
        "bk": np.asarray(bk, np.float32).astype(np.float16).reshape(1, D),
        "bv": np.asarray(bv, np.float32).astype(np.float16).reshape(1, D),
        "bo": np.asarray(bo, np.float32).astype(np.float16).reshape(1, D),
        "ident": np.eye(128, dtype=np.float32),
        "ones": np.ones((1, D), np.float16),
    }
    in_maps = []
    for c in range(N_CORES):
        b, th = c // 2, c % 2
        in_maps.append({
            "x": np.ascontiguousarray(x[b, th * TS:(th + 1) * TS, :]),
            "mem": np.ascontiguousarray(mem[b]),
            **common,
        })
    return in_maps


def assemble(results):
    out = np.empty((B, T, D), np.float32)
    for c in range(N_CORES):
        b, th = c // 2, c % 2
        out[b, th * TS:(th + 1) * TS, :] = results[c]["out"]
    return out


def kernel(**inputs):
    from concourse.bass_utils import run_bass_kernel_spmd

    nc = _get_nc()
    in_maps = build_in_maps(**inputs)
    res = run_bass_kernel_spmd(nc, in_maps, list(range(N_CORES)))
    _CACHE["last_res"] = res
    return assemble(res.results)

